# revision 26
# baseline (speedup 1.0000x reference)
"""Trainium2 Bass kernel for nn_Attention_56916906606885 (topk channel masking).

Reference computation (per sample b of 32):
  avg[c] = mean(x[b,c,:,:]); mx[c] = max(x[b,c,:,:])          # [512]
  z = conv1d(avg,w,pad=1) + conv1d(mx,w,pad=1)                 # [512] logits
  scores = sigmoid(z)
  top K=256 channels by score, re-sorted by ascending channel index
  out[b,j] = scores[sidx[j]] * x[b, sidx[j]]                   # [256,56,56]

Design (8 NeuronCores, data-parallel over batch, 4 samples/core):
  One pass over x: stream [128,2,3136] double-channel tiles into SBUF.
    - per-channel max on VectorE (tensor_reduce; NOTE: accum-stage max
      via tensor_tensor_reduce or tensor_scalar+accum_out CRASHES the
      TRN2 DVE exec unit — only add-accum works, so the max stays on
      the 1-elem/cycle tensor_reduce path)
    - per-channel sum on ScalarE (activation Copy + accum_out)
  z conv1d computed WITHOUT any DMA: the channel-axis partition shifts
  are done on TensorE with shift matrices (shl = S_dn @ comb plus a
  rank-1 wrap-term accumulated into the same PSUM tile). DMA-based
  shifts are poison: HWDGE rings share one TPB-level DGE FIFO, so a
  tiny shift DMA queues behind megabytes of x loads (~30us stall).
  Selection WITHOUT sort: rank[i] = #{j : z[j] > z[i]} via fused
  tensor_scalar ops against a PE-broadcast z row; mask = rank < K;
  compacted output position = prefix-sum via PE matmuls.
  4-deep software pipeline  stats(s) | select(s-1) | scale(s-2) |
  scatter(s-3)  so each engine's in-order stream never waits on a
  cross-engine round-trip. Scales split ScalarE(11)/VectorE(5);
  GpSimd runs ONLY the indirect scatters (streaming ops there would
  hold the shared DVE/GpSimd SBUF port and block VectorE). Scatters
  are 2 per sample (offset AP [128,2] over a whole 2-tile buffer):
  same-tensor indirect DMAs serialize on completion, so fewer+bigger.
  Sigmoid + Copy activation tables are preloaded at t=0 so no table
  load lands mid-pipeline.
  Selection operates on the pre-sigmoid logit z (sigmoid is monotonic;
  min boundary gap 2.9e-5 in logit space vs ~1e-6 arithmetic noise),
  so the table-based sigmoid only affects the output scaling.
"""

import sys

for _p in ("/opt/trn_rl_repo",):
    if _p not in sys.path:
        sys.path.insert(0, _p)

import numpy as np

import concourse.bass as bass
import concourse.bacc as bacc
import concourse.tile as tile
from concourse import mybir
from concourse.bass_utils import run_bass_kernel_spmd

F32 = mybir.dt.float32
I32 = mybir.dt.int32
AF = mybir.ActivationFunctionType
OP = mybir.AluOpType

B, C, H, W = 32, 512, 56, 56
HW = H * W  # 3136
HALF = HW // 2  # 1568
K = 256
NCORES = 8
SPB = B // NCORES  # 4 samples per core
P = 128
NT = C // P  # 4 channel tiles per sample
NH = NT // 2  # 2 double-tiles per sample
FLAT_IN = SPB * C  # 2048 rows per core
FLAT_OUT = SPB * K  # 1024 rows per core
BIG = 65536.0  # OOB marker for unselected channels (> any valid row index)
FMIN = -3.0e38  # init value for the max reduction

_CACHE = {}


def build_nc(finalize=True):
    nc = bacc.Bacc()
    x = nc.declare_dram_parameter("x", [FLAT_IN, HW], F32, isOutput=False)
    wt = nc.declare_dram_parameter("w", [1, 3], F32, isOutput=False)
    outs = [
        nc.declare_dram_parameter(f"out{s}", [K, HW], F32, isOutput=True)
        for s in range(SPB)
    ]

    with tile.TileContext(nc) as tc:
        with (
            tc.tile_pool(name="xp", bufs=7) as xp,
            tc.tile_pool(name="small", bufs=1) as sp,
            tc.tile_pool(name="trash", bufs=1) as tp,
            tc.tile_pool(name="rows", bufs=3) as rp,
            tc.tile_pool(name="psum", bufs=2, space="PSUM") as pp,
            tc.tile_pool(name="psum2", bufs=2, space="PSUM") as pp2,
            tc.tile_pool(name="psum3", bufs=1, space="PSUM") as pp3,
        ):
            # ---------- one-time constants ----------
            w_bc = sp.tile([P, 3], F32, tag="w_bc")
            nc.scalar.dma_start(w_bc[:, :], wt[0:1, :].to_broadcast([P, 3]))

            onesPC = sp.tile([P, C], F32, tag="onesPC")
            nc.vector.memset(onesPC[:, :], 1.0)
            ones128 = sp.tile([P, P], F32, tag="ones128")
            nc.vector.memset(ones128[:, :], 1.0)

            # preload ACT tables (Sigmoid + Copy) so no mid-pipeline load
            tbl = sp.tile([P, 1], F32, tag="tbl")
            nc.scalar.activation(tbl[:, 0:1], onesPC[:, 0:1], AF.Sigmoid)
            nc.scalar.activation(tbl[:, 0:1], onesPC[:, 0:1], AF.Copy)

            # ident[p, i] = [i == p]
            ident = sp.tile([P, P], F32, tag="ident")
            nc.gpsimd.affine_select(
                ident[:, :], onesPC[:, 0:P], [[-1, P]], OP.is_equal, 0.0,
                base=0, channel_multiplier=1,
            )
            # L128[j, m] = [j <= m]  (inclusive lower prefix)
            L128 = sp.tile([P, P], F32, tag="L128")
            nc.gpsimd.affine_select(
                L128[:, :], onesPC[:, 0:P], [[1, P]], OP.is_ge, 0.0,
                base=0, channel_multiplier=-1,
            )
            # shift matrices: S_dn[k,i]=[i==k+1], S_up[k,i]=[i==k-1]
            S_dn = sp.tile([P, P], F32, tag="S_dn")
            nc.gpsimd.affine_select(
                S_dn[:, :], onesPC[:, 0:P], [[-1, P]], OP.is_equal, 0.0,
                base=1, channel_multiplier=1,
            )
            S_up = sp.tile([P, P], F32, tag="S_up")
            nc.gpsimd.affine_select(
                S_up[:, :], onesPC[:, 0:P], [[-1, P]], OP.is_equal, 0.0,
                base=-1, channel_multiplier=1,
            )
            # wrap selectors: W_shl[k,i]=[k==127 && i==0] (unique root of
            # k + 128*i - 127 == 0), W_shr[k,i]=[k==0 && i==127]
            # (unique root of 128*k - i + 127 == 0)
            W_shl = sp.tile([P, P], F32, tag="W_shl")
            nc.gpsimd.affine_select(
                W_shl[:, :], onesPC[:, 0:P], [[P, P]], OP.is_equal, 0.0,
                base=-(P - 1), channel_multiplier=1,
            )
            W_shr = sp.tile([P, P], F32, tag="W_shr")
            nc.gpsimd.affine_select(
                W_shr[:, :], onesPC[:, 0:P], [[-1, P]], OP.is_equal, 0.0,
                base=P - 1, channel_multiplier=P,
            )
            # onehot4_t[k, m] = [k == t]
            onehot4 = sp.tile([SPB, P * NT], F32, tag="onehot4")
            for t in range(NT):
                nc.gpsimd.affine_select(
                    onehot4[0:NT, t * P : (t + 1) * P],
                    onesPC[0:NT, 0:P],
                    [[0, P]],
                    OP.is_equal,
                    0.0,
                    base=-t,
                    channel_multiplier=1,
                )
            sum_col = sp.tile([P, SPB * NT], F32, tag="sum_col")
            mx_col = sp.tile([P, SPB * NT], F32, tag="mx_col")
            comb_col = sp.tile([P, SPB * NT], F32, tag="comb_col")
            z_col = sp.tile([P, SPB * NT], F32, tag="z_col")
            score_col = sp.tile([P, SPB * NT], F32, tag="score_col")
            rank_col = sp.tile([P, SPB * NT], F32, tag="rank_col")
            m_col = sp.tile([P, SPB * NT], F32, tag="m_col")
            offf_col = sp.tile([P, SPB * NT], F32, tag="offf_col")
            offi_col = sp.tile([P, SPB * NT], I32, tag="offi_col")

            # full-size op outputs nobody reads
            trash_act = tp.tile([P, HW], F32, tag="trash_act")
            trash_rank = tp.tile([P, C], F32, tag="trash_rank")

            xt_of = {}

            def tview(s, t):
                """[128, 3136] view of channel-tile t inside its 2-tile buffer."""
                return xt_of[s][t // 2][:, t % 2, :]

            def phase_a(s):
                """Load sample s (two 2-tile DMAs); per-channel sum + max."""
                cols = slice(s * NT, (s + 1) * NT)
                xt = []
                for h in range(NH):
                    r0 = s * C + h * 2 * P
                    xth = xp.tile([P, 2, HW], F32, tag="xt")
                    nc.sync.dma_start(
                        xth[:, :, :],
                        x[r0 : r0 + 2 * P, :].rearrange("(t p) f -> p t f", t=2),
                    )
                    xt.append(xth)
                xt_of[s] = xt
                for t in range(NT):
                    col = slice(s * NT + t, s * NT + t + 1)
                    nc.scalar.activation(
                        trash_act[:, :], tview(s, t), AF.Copy,
                        accum_out=sum_col[:, col],
                    )
                    if t % 2 == 1:
                        h = t // 2
                        nc.vector.tensor_reduce(
                            mx_col[:, s * NT + 2 * h : s * NT + 2 * h + 2],
                            xt[h][:, :, :],
                            axis=mybir.AxisListType.X,
                            op=OP.max,
                        )
                nc.vector.scalar_tensor_tensor(
                    out=comb_col[:, cols],
                    in0=sum_col[:, cols],
                    scalar=1.0 / HW,
                    op0=OP.mult,
                    in1=mx_col[:, cols],
                    op1=OP.add,
                )

            def phase_b(s):
                """z (conv via shifted cols), rank, mask, offsets. No ScalarE."""
                cols = slice(s * NT, (s + 1) * NT)
                z4s = rp.tile([NT, P], F32, tag="z4s")

                # z[c] = w0*comb[c-1] + w1*comb[c] + w2*comb[c+1] in column
                # form; neighbor channels via PE shift matmuls (no DMA).
                # shl = S_dn @ comb, + e127-row wrap into cols 1..3
                shl_p = pp3.tile([P, NT], F32, tag="shl_p")
                nc.tensor.matmul(
                    out=shl_p[:, 0:NT], lhsT=S_dn[:, :], rhs=comb_col[:, cols],
                    start=True, stop=False,
                )
                nc.tensor.matmul(
                    out=shl_p[:, 1:NT], lhsT=W_shl[:, :],
                    rhs=comb_col[:, s * NT : s * NT + NT - 1],
                    start=False, stop=True,
                )
                shr_p = pp3.tile([P, NT], F32, tag="shr_p")
                nc.tensor.matmul(
                    out=shr_p[:, 0:NT], lhsT=S_up[:, :], rhs=comb_col[:, cols],
                    start=True, stop=False,
                )
                nc.tensor.matmul(
                    out=shr_p[:, 0 : NT - 1], lhsT=W_shr[:, :],
                    rhs=comb_col[:, s * NT + 1 : s * NT + NT],
                    start=False, stop=True,
                )
                nc.vector.tensor_scalar(
                    z_col[:, cols], shr_p[:, :], w_bc[:, 2:3], None, op0=OP.mult
                )
                nc.vector.scalar_tensor_tensor(
                    out=z_col[:, cols],
                    in0=comb_col[:, cols],
                    scalar=w_bc[:, 1:2],
                    op0=OP.mult,
                    in1=z_col[:, cols],
                    op1=OP.add,
                )
                nc.vector.scalar_tensor_tensor(
                    out=z_col[:, cols],
                    in0=shl_p[:, :],
                    scalar=w_bc[:, 0:1],
                    op0=OP.mult,
                    in1=z_col[:, cols],
                    op1=OP.add,
                )

                # z tile-rows [NT, 128] then broadcast to all partitions
                z4p = pp.tile([NT, P], F32, tag="z4p")
                nc.tensor.transpose(z4p[:, :], z_col[:, cols], ident[:, :])
                nc.vector.tensor_copy(z4s[:, :], z4p[:, :])
                zbp = pp.tile([P, C], F32, tag="zbp")
                for t in range(NT):
                    nc.tensor.matmul(
                        out=zbp[:, t * P : (t + 1) * P],
                        lhsT=onehot4[0:NT, t * P : (t + 1) * P],
                        rhs=z4s[:, :],
                        start=True,
                        stop=True,
                    )
                # rank[i] = #{j : z[j] > z[i]}
                for t in range(NT):
                    col = slice(s * NT + t, s * NT + t + 1)
                    nc.vector.tensor_scalar(
                        trash_rank[:, :],
                        zbp[:, :],
                        z_col[:, col],
                        None,
                        op0=OP.is_gt,
                        op1=OP.add,
                        accum_out=rank_col[:, col],
                    )
                nc.vector.tensor_scalar(
                    m_col[:, cols], rank_col[:, cols], float(K), None, op0=OP.is_lt
                )
                # inclusive prefix of mask, straight to column form:
                # incl_col[:, t] = sum_{k<t} ones128 @ m_k + L128 @ m_t
                incl_colp = pp2.tile([P, NT], F32, tag="colp")
                nc.tensor.matmul(
                    out=incl_colp[:, 0:NT],
                    lhsT=L128[:, :],
                    rhs=m_col[:, cols],
                    start=True,
                    stop=False,
                )
                for k in range(NT - 1):
                    nc.tensor.matmul(
                        out=incl_colp[:, k + 1 : NT],
                        lhsT=ones128[:, :],
                        rhs=m_col[
                            :, s * NT + k : s * NT + k + 1
                        ].to_broadcast([P, NT - 1 - k]),
                        start=False,
                        stop=(k == NT - 2),
                    )
                # off = incl + BIG + m*(s*K - 1 - BIG); unselected stay > bounds
                nc.vector.scalar_tensor_tensor(
                    out=offf_col[:, cols],
                    in0=m_col[:, cols],
                    scalar=float(-1 - BIG),
                    op0=OP.mult,
                    in1=incl_colp[:, :],
                    op1=OP.add,
                )
                nc.vector.tensor_scalar(
                    offi_col[:, cols], offf_col[:, cols], BIG, None, op0=OP.add
                )

            def phase_b2(s, pos):
                """Sigmoid + scale the four tiles (ScalarE/VectorE split)."""
                cols = slice(s * NT, (s + 1) * NT)
                nc.scalar.activation(score_col[:, cols], z_col[:, cols], AF.Sigmoid)
                for t in range(NT):
                    col = slice(s * NT + t, s * NT + t + 1)
                    on_act = (t in (0, 2)) or (
                        t == 1 and 0 < pos < SPB - 1
                    )
                    if on_act:
                        nc.scalar.activation(
                            tview(s, t), tview(s, t), AF.Copy,
                            scale=score_col[:, col],
                        )
                    else:
                        nc.vector.tensor_scalar(
                            tview(s, t), tview(s, t), score_col[:, col], None,
                            op0=OP.mult,
                        )

            def phase_c(s):
                """Scatter selected (already scaled) rows to DRAM."""
                for t in (1, 0, 3, 2):
                    col = slice(s * NT + t, s * NT + t + 1)
                    nc.gpsimd.indirect_dma_start(
                        out=outs[s][:, :],
                        out_offset=bass.IndirectOffsetOnAxis(
                            ap=offi_col[:, col], axis=0
                        ),
                        in_=tview(s, t),
                        in_offset=None,
                        bounds_check=K - 1,
                        oob_is_err=False,
                    )
                xt_of.pop(s)

            # 4-deep pipeline  scatter | scale | select | stats, except
            # the first sample runs depth-3 (scale at step 1, scatter at
            # step 2) so the serialized scatter chain starts ~12us
            # earlier. Samples are processed in order (0,1,3,2): the 7-buf
            # pool stalls the 4th-loaded sample behind the first frees, so
            # the stalled one should be the sample processed LAST.
            seq = (0, 1, 3, 2)
            for step in range(SPB + 3):
                if step == 2:
                    phase_c(seq[0])
                elif step >= 4:
                    phase_c(seq[step - 3])
                if 1 <= step <= SPB:
                    phase_b(seq[step - 1])
                if step == 1:
                    phase_b2(seq[0], 0)
                elif 3 <= step <= SPB + 1:
                    phase_b2(seq[step - 2], step - 2)
                if step < SPB:
                    phase_a(seq[step])
    if finalize:
        nc.finalize()
    return nc


def kernel(x: np.ndarray, w: np.ndarray) -> np.ndarray:
    assert x.shape == (B, C, H, W) and w.shape == (1, 1, 3)
    if "nc" not in _CACHE:
        _CACHE["nc"] = build_nc()
    nc = _CACHE["nc"]

    xs = np.ascontiguousarray(x, dtype=np.float32).reshape(NCORES, FLAT_IN, HW)
    ws = np.ascontiguousarray(w, dtype=np.float32).reshape(1, 3)
    in_maps = [{"x": xs[i], "w": ws} for i in range(NCORES)]
    res = run_bass_kernel_spmd(nc, in_maps, core_ids=list(range(NCORES)))
    full = []
    for r in res.results:
        full.extend(
            np.asarray(r[f"out{s}"]).reshape(1, K, H, W) for s in range(SPB)
        )
    return np.concatenate(full, axis=0)


if __name__ == "__main__":
    xin = np.random.randn(B, C, H, W).astype(np.float32)
    win = np.random.randn(1, 1, 3).astype(np.float32)
    o = kernel(xin, win)
    print("kernel out", o.shape, o.dtype, float(np.abs(o).max()))
